# revision 1
# baseline (speedup 1.0000x reference)
"""GCN layer kernel for 8 Trainium2 NeuronCores.

out = segment_sum(edge_vals * (features @ W)[edge_src], edge_dst) + bias
    = segment_sum(edge_vals * features[edge_src], edge_dst) @ W + bias
      (W is shared across nodes, so aggregation commutes with the matmul)

Strategy (graph/data parallel per sharding hint):
- Destination nodes are sharded across 8 cores (12500 per core).
- Host prep per core: sort local dst nodes by degree, pack them into
  windows of 128 (one SBUF partition per dst). For slot k of a window,
  one indirect DMA gathers "the k-th neighbor's feature row" for all
  128 dsts; DVE accumulates agg += val * row in fp32.
- Degree sorting makes slot counts nearly uniform inside a window, so
  padding (idx=0, val=0) is small.
- Per window: PE transposes agg, multiplies by W (replicated), adds bias,
  DMA to DRAM. Host un-permutes rows to the original node order.

All FLOPs (scale, segment-sum, weight matmul, bias) run on device; the
host only computes index schedules / permutations and re-orders rows.
"""
import os
import sys
from contextlib import ExitStack

import numpy as np

_REPO = "/opt/trn_rl_repo"
if _REPO not in sys.path:
    sys.path.insert(0, _REPO)

N_NODES = 100000
N_EDGES = 3200000
DIM = 128
N_CORES = 8
P = 128
SHARD = N_NODES // N_CORES  # 12500
N_WIN = (SHARD + P - 1) // P  # 98 windows/core
SHARD_PAD = N_WIN * P  # 12544


def _host_schedule(edge_src, edge_dst, edge_vals):
    """Build the per-core degree-grid schedule.

    Returns (slots_per_window [N_WIN], per-core dicts with idx_stream,
    val_stream, perm) where perm[i] = original local dst row stored at
    padded slot i.
    """
    core = edge_dst // SHARD
    cores = []
    deg_sorted_all = []
    for c in range(N_CORES):
        m = core == c
        src_c = edge_src[m]
        dst_c = edge_dst[m] - c * SHARD
        val_c = edge_vals[m]
        # sort edges by local dst so each dst's edges are contiguous
        order = np.argsort(dst_c, kind="stable")
        src_c, dst_c, val_c = src_c[order], dst_c[order], val_c[order]
        deg = np.bincount(dst_c, minlength=SHARD)
        starts = np.concatenate([[0], np.cumsum(deg)[:-1]])
        # degree-descending dst order -> windows have uniform degrees
        perm = np.argsort(-deg, kind="stable")  # padded slot i <- dst perm[i]
        cores.append(
            {"src": src_c, "val": val_c, "deg": deg, "starts": starts, "perm": perm}
        )
        deg_sorted_all.append(deg[perm])
    # slots per window: max over cores of max degree within the window
    slots = np.zeros(N_WIN, dtype=np.int64)
    for w in range(N_WIN):
        lo = w * P
        hi = min(lo + P, SHARD)
        mx = max(int(d[lo:hi].max()) if hi > lo else 0 for d in deg_sorted_all)
        slots[w] = max(mx, 1)  # at least one slot so every window writes bias
    total_cols = int(slots.sum())
    for c in range(N_CORES):
        cc = cores[c]
        idx_stream = np.zeros((P, total_cols), dtype=np.int32)
        val_stream = np.zeros((P, total_cols), dtype=np.float32)
        col = 0
        for w in range(N_WIN):
            lo = w * P
            for k in range(int(slots[w])):
                # partition p of this window handles local dst cc.perm[lo+p]
                for_p = lo + np.arange(P)
                valid = for_p < SHARD
                d = cc["perm"][np.minimum(for_p, SHARD - 1)]
                has = valid & (cc["deg"][d] > k)
                e = cc["starts"][d] + k
                idx_stream[has, col + k] = cc["src"][e[has]]
                val_stream[has, col + k] = cc["val"][e[has]]
            col += int(slots[w])
        cc["idx_stream"] = idx_stream
        cc["val_stream"] = val_stream
    return slots, total_cols, cores


def _indirect_gather_q(nc, out_ap, in_ap, offset_ap, queue_name):
    """Clone of bass indirect_dma_start (gather form) with queue override."""
    import concourse.bass as bass
    from concourse import mybir

    eng = nc.gpsimd
    out_l = eng.lower_ap_dma(out_ap, for_indirect_dma=True)
    in_l = eng.lower_ap_dma(in_ap, for_indirect_dma=True)
    assert len(in_l) == 1 and len(out_l) == 1
    off_l = eng.lower_ap_dma(offset_ap)
    assert len(off_l) == 1
    ap_shape = in_ap.shape
    coef = 1
    for i in range(1, len(ap_shape)):
        coef *= ap_shape[i]
    in_l[0].dynamic_ap_info = mybir.DynamicAccessPatternInfo(
        c=0,
        actual_ap=out_ap.ap,
        indirect_dim_max_index=ap_shape[0],
        offset_expr=[
            mybir.DynamicAccessPatternOffsetExpr(
                coef=coef,
                aff_expr=mybir.DynamicAccessPatternOffsetExprAffExpr(
                    kind="IndirectArgId", arg_id=1
                ),
            )
        ],
    )
    in_l.append(off_l[0])
    return eng.add_instruction(
        mybir.InstDMACopy(
            name=nc.get_next_instruction_name(),
            queue=queue_name,
            mode="Copy",
            ins=in_l,
            outs=out_l,
            oob_is_err=True,
            cce_op=mybir.AluOpType.bypass,
        )
    )


N_QUEUES = int(os.environ.get("GCN_QUEUES", "1"))


def _build_nc(slots, total_cols):
    import concourse.bass as bass
    import concourse.tile as tile
    from concourse import bacc, mybir

    nc = bacc.Bacc(
        "TRN2", target_bir_lowering=False, debug=False, num_devices=N_CORES,
        num_swdge_queues=N_QUEUES,
    )
    feat_t = nc.dram_tensor("features", [N_NODES, DIM], mybir.dt.float32, kind="ExternalInput")
    idx_t = nc.dram_tensor("idx_stream", [P, total_cols], mybir.dt.int32, kind="ExternalInput")
    val_t = nc.dram_tensor("val_stream", [P, total_cols], mybir.dt.float32, kind="ExternalInput")
    w_t = nc.dram_tensor("weight", [DIM, DIM], mybir.dt.float32, kind="ExternalInput")
    bias_t = nc.dram_tensor("bias_tile", [P, DIM], mybir.dt.float32, kind="ExternalInput")
    ident_t = nc.dram_tensor("identity", [P, P], mybir.dt.float32, kind="ExternalInput")
    out_t = nc.dram_tensor("outp", [N_WIN, P, DIM], mybir.dt.float32, kind="ExternalOutput")

    with tile.TileContext(nc) as tc:
        with ExitStack() as ctx:
            const = ctx.enter_context(tc.tile_pool(name="const", bufs=1))
            gpool = ctx.enter_context(tc.tile_pool(name="gather", bufs=16))
            tpool = ctx.enter_context(tc.tile_pool(name="tmp", bufs=16))
            apool = ctx.enter_context(tc.tile_pool(name="agg", bufs=4))
            opool = ctx.enter_context(tc.tile_pool(name="outw", bufs=3))
            tppool = ctx.enter_context(tc.tile_pool(name="aggt", bufs=3))
            pspool = ctx.enter_context(
                tc.tile_pool(name="psum", bufs=4, space="PSUM")
            )

            idx_all = const.tile([P, total_cols], mybir.dt.int32)
            nc.sync.dma_start(idx_all[:], idx_t[:])
            val_all = const.tile([P, total_cols], mybir.dt.float32)
            nc.sync.dma_start(val_all[:], val_t[:])
            w_tile = const.tile([DIM, DIM], mybir.dt.float32)
            nc.sync.dma_start(w_tile[:], w_t[:])
            bias_tile = const.tile([P, DIM], mybir.dt.float32)
            nc.sync.dma_start(bias_tile[:], bias_t[:])
            ident = const.tile([P, P], mybir.dt.float32)
            nc.sync.dma_start(ident[:], ident_t[:])

            col = 0
            for w in range(N_WIN):
                agg = apool.tile([P, DIM], mybir.dt.float32)
                for k in range(int(slots[w])):
                    g = gpool.tile([P, DIM], mybir.dt.float32)
                    if N_QUEUES > 1:
                        qn = (col + k) % N_QUEUES
                        _indirect_gather_q(
                            nc, g[:], feat_t[:],
                            idx_all[:, col + k:col + k + 1],
                            f"qPoolDynamic{qn or ''}",
                        )
                    else:
                        nc.gpsimd.indirect_dma_start(
                            out=g[:],
                            out_offset=None,
                            in_=feat_t[:],
                            in_offset=bass.IndirectOffsetOnAxis(
                                ap=idx_all[:, col + k:col + k + 1], axis=0
                            ),
                        )
                    # scale on ScalarE (ACT): keeps DVE 2-port ops off the
                    # SBUF port GpSimd needs for SWDGE descriptor rings
                    if k == 0:
                        nc.scalar.activation(
                            agg[:], g[:], mybir.ActivationFunctionType.Copy,
                            scale=val_all[:, col + k:col + k + 1],
                        )
                    else:
                        t = tpool.tile([P, DIM], mybir.dt.float32)
                        nc.scalar.activation(
                            t[:], g[:], mybir.ActivationFunctionType.Copy,
                            scale=val_all[:, col + k:col + k + 1],
                        )
                        nc.vector.tensor_add(agg[:], agg[:], t[:])
                col += int(slots[w])

                # aggT = transpose(agg) : [k(in), d]
                ps_t = pspool.tile([P, P], mybir.dt.float32)
                nc.tensor.transpose(out=ps_t[:], in_=agg[:], identity=ident[:])
                agg_tr = tppool.tile([P, P], mybir.dt.float32)
                nc.vector.tensor_copy(agg_tr[:], ps_t[:])
                # out_w[d, f] = sum_k agg[d, k] W[k, f]
                ps_o = pspool.tile([P, DIM], mybir.dt.float32)
                nc.tensor.matmul(
                    out=ps_o[:], lhsT=agg_tr[:], rhs=w_tile[:], start=True, stop=True
                )
                ow = opool.tile([P, DIM], mybir.dt.float32)
                nc.vector.tensor_add(ow[:], ps_o[:], bias_tile[:])
                nc.sync.dma_start(out_t[w], ow[:])
    nc.compile()
    return nc


def kernel(features, edge_src, edge_dst, edge_vals, weight, bias):
    features = np.ascontiguousarray(np.asarray(features), dtype=np.float32)
    edge_src = np.asarray(edge_src).astype(np.int64)
    edge_dst = np.asarray(edge_dst).astype(np.int64)
    edge_vals = np.asarray(edge_vals).astype(np.float32)
    weight = np.asarray(weight).astype(np.float32)
    bias = np.asarray(bias).astype(np.float32)

    slots, total_cols, cores = _host_schedule(edge_src, edge_dst, edge_vals)
    nc = _build_nc(slots, total_cols)

    from concourse.bass_utils import run_bass_kernel_spmd

    bias_tile = np.tile(bias[None, :], (P, 1)).astype(np.float32)
    ident = np.eye(P, dtype=np.float32)
    in_maps = []
    for c in range(N_CORES):
        in_maps.append(
            {
                "features": features,
                "idx_stream": cores[c]["idx_stream"],
                "val_stream": cores[c]["val_stream"],
                "weight": weight,
                "bias_tile": bias_tile,
                "identity": ident,
            }
        )
    trace = os.environ.get("GCN_TRACE", "0") == "1"
    res = None
    for attempt in range(3):
        try:
            res = run_bass_kernel_spmd(
                nc, in_maps, core_ids=list(range(N_CORES)), trace=trace
            )
            break
        except Exception:
            if attempt == 2:
                raise
            import time as _time

            _time.sleep(15.0)  # transient device flakes recover across retries
    if trace:
        print(f"HW exec time: {res.exec_time_ns} ns")
        kernel.last_exec_time_ns = res.exec_time_ns

    out = np.empty((N_NODES, DIM), dtype=np.float32)
    for c in range(N_CORES):
        op = res.results[c]["outp"].reshape(SHARD_PAD, DIM)
        perm = cores[c]["perm"]  # padded slot i holds local dst perm[i]
        local = np.empty((SHARD, DIM), dtype=np.float32)
        local[perm] = op[:SHARD]
        out[c * SHARD:(c + 1) * SHARD] = local
    return out


kernel.last_exec_time_ns = None

